# revision 17
# baseline (speedup 1.0000x reference)
"""DISCO (discrete-continuous) spherical conv encoder on 8 Trainium2 cores.

Strategy: output-latitude sharding (361 rows -> ~46/core), no collectives.
Host folds weight[o,c,k] x psi[k,h,l,d] x quad_w[lat_idx[h,l]] into per-h
matmul coefficients; device does per-latitude-group matmuls with PSUM
accumulation over the 9 longitude shifts (stride-2 rhs APs give the
PSCALE=2 decimation for free; a 4-col halo handles the longitude wrap).
"""
import os
import numpy as np
import ml_dtypes

B, CIN, COUT = 1, 16, 16
HIN, WIN = 721, 1440
HOUT, WOUT = 361, 720
KBAS, NL, ND = 9, 7, 9
NCORES = 8
HPC = 46          # valid output rows per core (last core: 39)
GRP = 8           # output rows per group
NG = 6            # groups per core (8*6=48 slots >= 46)
HBLK = NG * GRP   # 48
HALO = ND // 2    # 4
WROW = WIN + 2 * HALO  # 1448
NCHUNKS = ((0, 512), (512, WOUT - 512))  # psum-bank-aligned N split
# v3 order: short chunk first so the pulled-ahead LDWEIGHTS of the next
# (tau, d) group (~99ns) hides fully under the long 512-col matmul
NCHUNKS3 = ((512, WOUT - 512), (0, 512))

# ---- v2 sliding-window scheme constants
HPC2 = 48         # output rows per core (48*8=384 >= 361); 48%8==0 keeps the
                  # block->slot mapping identical on every core (SPMD)
NT = 14           # aligned 8-input-row blocks per core (incl leading dummy)
NPS = 4           # rotating PSUM accumulators (4 x 2 banks = all 8 banks)

# ---- v3: like v2 but blocks offset so no dummy block is needed
NT3 = 13          # 8-input-row blocks per core, base input row 96*i - 3

_cache = {}
last_result = None


def _build_nc(RG, KT, kparts, dt_in):
    import concourse.bass as bass
    import concourse.bacc as bacc
    import concourse.mybir as mybir
    from concourse import tile

    nc = bacc.Bacc("TRN2", target_bir_lowering=False, debug=False,
                   num_devices=NCORES)
    xr = nc.declare_dram_parameter("xr", [CIN, NG * RG, WIN], dt_in,
                                   isOutput=False)
    w2 = nc.declare_dram_parameter("w2", [NG, 128, ND * KT * 128], dt_in,
                                   isOutput=False)
    y = nc.declare_dram_parameter("y", [COUT, HBLK, WOUT], mybir.dt.float32,
                                  isOutput=True)
    xr_t = xr.ap().transpose([1, 0, 2])  # [row, c, w]
    y_t = y.ap().transpose([1, 0, 2])    # [h, o, w]

    with tile.TileContext(nc) as tc:
        with (
            tc.tile_pool(name="rbp", bufs=2) as rbp,
            tc.tile_pool(name="w2p", bufs=2) as w2p,
            tc.tile_pool(name="psp", bufs=2, space="PSUM") as psp,
            tc.tile_pool(name="outp", bufs=2) as outp,
        ):
            for g in range(NG):
                w2t = w2p.tile([128, ND * KT * 128], dt_in, tag="w2")
                nc.sync.dma_start(out=w2t[:, :], in_=w2.ap()[g])
                rbs = []
                for kt in range(KT):
                    nrows = (kparts[kt] + 15) // 16  # rows in this k-tile
                    rb = rbp.tile([128, WROW], dt_in, tag=f"rb{kt}")
                    np_ = nrows * CIN
                    r0 = g * RG + 8 * kt  # first xr row of this tile
                    # body + wrap halos; partitions = (row, c), src 3D
                    for dst_c0, src_c0, ncol in (
                        (HALO, 0, WIN),
                        (0, WIN - HALO, HALO),
                        (HALO + WIN, 0, HALO),
                    ):
                        nc.sync.dma_start(
                            out=rb[0:np_, dst_c0:dst_c0 + ncol],
                            in_=xr_t[r0:r0 + nrows, :, src_c0:src_c0 + ncol])
                    rbs.append(rb)
                pss = [psp.tile([128, nw], mybir.dt.float32, tag=f"ps{ci}",
                                name=f"ps{ci}_{g}")
                       for ci, (w0, nw) in enumerate(NCHUNKS)]
                for d in range(ND):
                    for kt in range(KT):
                        kp = kparts[kt]
                        lhsT = w2t[0:kp, (d * KT + kt) * 128:
                                   (d * KT + kt) * 128 + 128]
                        first = d == 0 and kt == 0
                        last = d == ND - 1 and kt == KT - 1
                        for ci, (w0, nw) in enumerate(NCHUNKS):
                            c0 = d + 2 * w0
                            nc.tensor.matmul(
                                pss[ci][:, :], lhsT,
                                rbs[kt][0:kp, c0:c0 + 2 * nw:2],
                                start=first, stop=last)
                stage = outp.tile([128, WOUT], mybir.dt.float32, tag="stage",
                                  name=f"stage_{g}")
                for ci, (w0, nw) in enumerate(NCHUNKS):
                    nc.vector.tensor_copy(out=stage[:, w0:w0 + nw],
                                          in_=pss[ci][:, :])
                nc.sync.dma_start(
                    out=y_t[g * GRP:(g + 1) * GRP, :, :], in_=stage[:, :])
    nc.compile()
    return nc


def _build_nc_v2(dt_in):
    """Sliding-window scheme: one K=128 matmul tile per aligned 8-input-row
    block x 9 lon shifts, accumulating into a rotating set of 4 PSUM tiles
    with output slot = (local output row) mod 8.  After block tau, output
    rows 4*tau-5 .. 4*tau-2 (local) are complete: rows spanning two blocks
    are summed from two PSUM tiles on the Vector engine, single-block rows
    are copied, and the result is DMA'd to y rows [4*tau, 4*tau+4)."""
    import concourse.bacc as bacc
    import concourse.mybir as mybir
    from concourse import tile

    nc = bacc.Bacc("TRN2", target_bir_lowering=False, debug=False,
                   num_devices=NCORES)
    xw = nc.declare_dram_parameter("xw", [NT, 128, WROW + ND * 128],
                                   dt_in, isOutput=False)
    y = nc.declare_dram_parameter("y", [4 * NT, COUT, WOUT],
                                  mybir.dt.float32, isOutput=True)

    with tile.TileContext(nc) as tc:
        with (
            tc.tile_pool(name="xbp", bufs=3) as xbp,
            tc.tile_pool(name="psp", bufs=1, space="PSUM") as psp,
            tc.tile_pool(name="stp", bufs=4) as stp,
        ):
            P = [psp.tile([128, WOUT], mybir.dt.float32, tag=f"P{j}",
                          name=f"P{j}") for j in range(NPS)]
            for tau in range(NT):
                xb = xbp.tile([128, WROW + ND * 128], dt_in, tag="xb",
                              name=f"xb{tau}")
                # two issues -> two HW queues -> ~half the load latency
                nc.sync.dma_start(out=xb[0:64, :], in_=xw.ap()[tau][0:64])
                nc.sync.dma_start(out=xb[64:128, :], in_=xw.ap()[tau][64:128])
                ps = P[tau % NPS]
                for d in range(ND):
                    lhsT = xb[:, WROW + d * 128:WROW + (d + 1) * 128]
                    for w0, nw in NCHUNKS:
                        c0 = d + 2 * w0
                        nc.tensor.matmul(ps[:, w0:w0 + nw], lhsT,
                                         xb[:, c0:c0 + 2 * nw:2],
                                         start=(d == 0), stop=(d == ND - 1))
                # flush the 4 output rows finished by this block; with
                # slot = (h_local+1)%8 the flush set is one 64-partition
                # range at base 64 (even tau) / 0 (odd tau).  The j==3
                # (single-block) row's prev-tile contribution is exactly 0
                # (its weights there are zero), so one add covers all 4.
                st = stp.tile([128, WOUT], mybir.dt.float32, tag="st",
                              name=f"st{tau}")
                prev = P[(tau - 1) % NPS]
                p0 = 64 if tau % 2 == 0 else 0
                if tau == 0:
                    # dummy flush: no valid rows; keep the program uniform
                    nc.vector.tensor_copy(out=st[p0:p0 + 64, :],
                                          in_=ps[p0:p0 + 64, :])
                else:
                    # DVE reads at most one PSUM operand: stage prev via the
                    # (otherwise idle) Scalar engine.  The copy only depends
                    # on tile tau-1, so it overlaps this tile's matmuls.
                    cp = stp.tile([128, WOUT], mybir.dt.float32, tag="cp",
                                  name=f"cp{tau}")
                    nc.scalar.copy(out=cp[p0:p0 + 64, :],
                                   in_=prev[p0:p0 + 64, :])
                    nc.vector.tensor_add(out=st[p0:p0 + 64, :],
                                         in0=cp[p0:p0 + 64, :],
                                         in1=ps[p0:p0 + 64, :])
                nc.gpsimd.dma_start(out=y.ap()[4 * tau:4 * tau + 4],
                                    in_=st[p0:p0 + 64, :])
    # The tile exit splits each matmul into InstLdweights + InstMatmult.
    # The two N-chunk matmuls of each (tau, d) share a stationary operand:
    # drop the second (redundant) load when it carries no semaphore ops.
    nskip = 0
    for bb in nc.m.functions[0].blocks:
        prev_w = None
        dels = []
        for inst in bb.instructions:
            if type(inst).__name__ != "InstLdweights":
                continue
            w = str(inst.ins[0])
            if w == prev_w and not inst.has_wait() and not inst.has_update():
                dels.append(inst)
            prev_w = w
        for inst in dels:
            bb.instructions.remove(inst)
        nskip += len(dels)
    assert nskip >= NT * ND - 8, \
        f"expected ~{NT * ND} redundant LDWEIGHTS, got {nskip}"
    nc.compile()
    return nc


def _build_nc_v3(dt_in, warmup=True):
    """v3 sliding-window scheme: per-core input rows start at 96*i-3 so
    block tau's 8 rows serve exactly output rows hl in [4t-3, 4t+3]; the
    flush after tau covers hl in [4t-3, 4t] at slot (hl+3)%8 (one
    64-partition half, base 0 for even tau / 64 for odd).  Row 4t lies
    entirely in block tau, rows 4t-3..4t-1 straddle tau-1/tau and are
    summed from two PSUM tiles.  tau=0's flush has no prev tile and row 0
    (slot 3) is single-block, so it is a plain copy."""
    import concourse.bacc as bacc
    import concourse.mybir as mybir
    from concourse import tile

    nc = bacc.Bacc("TRN2", target_bir_lowering=False, debug=False,
                   num_devices=NCORES)
    xw = nc.declare_dram_parameter("xw", [NT3, 128, WROW + ND * 128],
                                   dt_in, isOutput=False)
    y = nc.declare_dram_parameter("y", [4 * NT3, COUT, WOUT],
                                  mybir.dt.bfloat16, isOutput=True)

    with tile.TileContext(nc) as tc:
        with (
            tc.tile_pool(name="xbp", bufs=NT3) as xbp,
            tc.tile_pool(name="psp", bufs=1, space="PSUM") as psp,
            # separate pools so st/cp each rotate over their own buffers
            # (shared-pool slots would halve the WAR reuse distance)
            tc.tile_pool(name="stp", bufs=5) as stp,
            tc.tile_pool(name="cpp", bufs=5) as cpp,
            tc.tile_pool(name="wup", bufs=1) as wup,
        ):
            P = [psp.tile([128, WOUT], mybir.dt.float32, tag=f"P{j}",
                          name=f"P{j}") for j in range(NPS)]
            if warmup:
                # HAM warmup: PE activity from program start until the
                # first x block lands (~11.4us) so the clock gate opens at
                # start+3.4us and never resets (an idle gap restarts the
                # busy window and the first real blocks run at 1.2GHz).
                wz = wup.tile([128, 640], dt_in, tag="wz")
                nc.gpsimd.memset(wz[:, :], 0.0)
                for i in range(11):
                    nc.tensor.matmul(P[NPS - 1][:, 0:512], wz[:, 0:128],
                                     wz[:, 128:640], start=True, stop=True)
            # Hoist all x-loads: Tile round-robins 8 DMA-completion sem
            # lanes over every DMA in program order; interleaving loads
            # with y-stores makes load dispatches wait on store completions
            # (which depend on the flush chain).  Issued back-to-back, the
            # loads only lane-wait on each other.
            xbs = []
            for tau in range(NT3):
                xb = xbp.tile([128, WROW + ND * 128], dt_in, tag="xb",
                              name=f"xb{tau}")
                if tau == 0:
                    # split so the d=0/d=1 groups start ~0.5us earlier
                    c1 = WROW + 2 * 128
                    nc.sync.dma_start(out=xb[:, 0:c1],
                                      in_=xw.ap()[tau][:, 0:c1])
                    nc.sync.dma_start(out=xb[:, c1:],
                                      in_=xw.ap()[tau][:, c1:])
                else:
                    nc.sync.dma_start(out=xb[:, :], in_=xw.ap()[tau])
                xbs.append(xb)
            for tau in range(NT3):
                xb = xbs[tau]
                ps = P[tau % NPS]
                for d in range(ND):
                    lhsT = xb[:, WROW + d * 128:WROW + (d + 1) * 128]
                    # zigzag: consecutive matmuls hit the same PSUM bank
                    # across group boundaries (bank switches cause PE
                    # micro-idles), and even-d order ends on the 512 chunk
                    # so the pulled-ahead LDWEIGHTS hides under it
                    chunks = NCHUNKS3 if d % 2 == 0 else NCHUNKS3[::-1]
                    for w0, nw in chunks:
                        c0 = d + 2 * w0
                        nc.tensor.matmul(ps[:, w0:w0 + nw], lhsT,
                                         xb[:, c0:c0 + 2 * nw:2],
                                         start=(d == 0), stop=(d == ND - 1))
                st = stp.tile([128, WOUT], mybir.dt.bfloat16, tag="st",
                              name=f"st{tau}")
                p0 = 0 if tau % 2 == 0 else 64
                nrow = 3 if tau == NT3 - 1 else 4  # hl=4*12 is never valid
                npart = 16 * nrow
                if tau == 0:
                    nc.scalar.copy(out=st[p0:p0 + npart, :],
                                   in_=ps[p0:p0 + npart, :])
                elif tau < NT3 - 1:
                    # ACT stages the CURRENT block's psum right after its
                    # matmuls; DVE adds the previous block's psum.  This
                    # frees P[tau%4] for block tau+4 via the immediate
                    # copy - the late reader (the add) touches P[(tau-1)%4]
                    # whose next writer is block tau+3, keeping the PSUM
                    # WAR release ~3 blocks ahead of the PE.
                    cp = cpp.tile([128, WOUT], mybir.dt.float32, tag="cp",
                                  name=f"cp{tau}")
                    prev = P[(tau - 1) % NPS]
                    nc.scalar.copy(out=cp[p0:p0 + npart, :],
                                   in_=ps[p0:p0 + npart, :])
                    nc.vector.tensor_add(out=st[p0:p0 + npart, :],
                                         in0=cp[p0:p0 + npart, :],
                                         in1=prev[p0:p0 + npart, :])
                else:
                    # last flush: stage the prev tile early (during this
                    # block's matmuls), and do the [512:720] bank first -
                    # its psum chunk is complete before the final 512-col
                    # matmul, so that add+store overlap the last matmul
                    cp = cpp.tile([128, WOUT], mybir.dt.float32, tag="cp",
                                  name=f"cp{tau}")
                    prev = P[(tau - 1) % NPS]
                    nc.scalar.copy(out=cp[p0:p0 + npart, :],
                                   in_=prev[p0:p0 + npart, :])
                    for w0, nw in ((512, WOUT - 512), (0, 512)):
                        nc.vector.tensor_add(
                            out=st[p0:p0 + npart, w0:w0 + nw],
                            in0=cp[p0:p0 + npart, w0:w0 + nw],
                            in1=ps[p0:p0 + npart, w0:w0 + nw])
                        nc.scalar.dma_start(
                            out=y.ap()[4 * tau:4 * tau + nrow]
                            [:, :, w0:w0 + nw],
                            in_=st[p0:p0 + npart, w0:w0 + nw])
                if tau < NT3 - 1:
                    nc.scalar.dma_start(out=y.ap()[4 * tau:4 * tau + nrow],
                                        in_=st[p0:p0 + npart, :])
    _dedup_ldweights(nc, NT3 * ND - 8)
    if os.environ.get("KERNEL_THIN", "1") == "1":
        _thin_pe_progress_sem(nc)
    nc.compile()
    return nc


def _thin_pe_progress_sem(nc):
    """Every matmul carries a +1 on the Tile PE-progress semaphore (~26ns
    of serialized EVT-register writes each).  Consumers only wait at a few
    thresholds, and a FIFO engine completes in order, so "first n done" ==
    "n-th done": keep the inc only on matmuls whose position is a waited
    threshold and renumber the waits to the threshold's rank."""
    import bass_rust

    f = nc.m.functions[0]
    from collections import Counter

    upd = Counter()
    for bb in f.blocks:
        for inst in bb.instructions:
            if type(inst).__name__ != "InstMatmult":
                continue
            si = inst.sync_info
            if si is None:
                continue
            for u in si.on_update:
                if u.update_mode == "sem-inc" and u.update_value == 1:
                    upd[u.id] += 1
    if not upd:
        return
    sem_id, n_mm = upd.most_common(1)[0]
    thresholds = set()
    ok = True
    for bb in f.blocks:
        for inst in bb.instructions:
            si = inst.sync_info
            if si is None:
                continue
            for w in si.on_wait:
                if w.id == sem_id:
                    if w.wait_mode != "sem-ge-imm" or w.wait_reg is not None:
                        ok = False
                    thresholds.add(w.wait_value)
    if not ok or not thresholds or max(thresholds) > n_mm:
        return
    T = sorted(thresholds)
    rank = {t: j + 1 for j, t in enumerate(T)}
    n = 0
    for bb in f.blocks:
        for inst in bb.instructions:
            si = inst.sync_info
            if si is None:
                continue
            touched = False
            new_upd = []
            for u in si.on_update:
                if (type(inst).__name__ == "InstMatmult"
                        and u.id == sem_id and u.update_mode == "sem-inc"):
                    n += 1
                    if n not in thresholds:
                        touched = True
                        continue  # drop the inc
                new_upd.append(u)
            new_wait = []
            for w in si.on_wait:
                if w.id == sem_id:
                    touched = True
                    w = bass_rust.SyncWait(
                        sync_type=w.sync_type, id=w.id, ant_name=w.ant_name,
                        wait_mode=w.wait_mode, wait_value=rank[w.wait_value])
                new_wait.append(w)
            if touched:
                inst.sync_info = bass_rust.SyncInfo(
                    on_wait=new_wait, on_update=new_upd)
    assert n == n_mm, (n, n_mm)


def _dedup_ldweights(nc, min_expected):
    """Drop back-to-back InstLdweights with an identical stationary
    operand (the Tile exit emits one per matmul; N-chunk pairs share)."""
    nskip = 0
    for bb in nc.m.functions[0].blocks:
        prev_w = None
        dels = []
        for inst in bb.instructions:
            if type(inst).__name__ != "InstLdweights":
                continue
            w = str(inst.ins[0])
            if w == prev_w and not inst.has_wait() and not inst.has_update():
                dels.append(inst)
            prev_w = w
        for inst in dels:
            bb.instructions.remove(inst)
        nskip += len(dels)
    assert nskip >= min_expected, \
        f"expected >= {min_expected} redundant LDWEIGHTS, got {nskip}"


def _v3_valid(lat):
    """v3 requires each output row's input rows to lie in the 1-2 blocks
    implied by its flush position (true for the structured equiangular
    lat_idx; arbitrary indices fall back to the generic group scheme)."""
    for i in range(NCORES):
        h0 = i * HPC2
        b0 = 96 * i - 3
        for h in range(h0, min(HOUT, h0 + HPC2)):
            hl = h - h0
            tf = (hl + 3) // 4
            rs = lat[h].astype(np.int64) - b0
            if rs.min() < 0 or rs.max() >= 8 * NT3:
                return False
            blocks = set(int(r) // 8 for r in rs)
            allowed = {tf} if tf == 0 else {tf - 1, tf}
            if not blocks <= allowed:
                return False
    return True


def _prepare_v3(x, psi, weight, quad_w, lat):
    use_f32 = os.environ.get("KERNEL_DTYPE", "bf16") == "f32"
    np_dt = np.float32 if use_f32 else ml_dtypes.bfloat16

    psi_q = psi.astype(np.float64) * \
        quad_w.astype(np.float64)[lat][None, :, :, None]
    W2 = np.einsum("ock,khld->hldco", weight.astype(np.float64),
                   psi_q).astype(np.float32)

    in_maps = []
    x0 = x[0]
    for i in range(NCORES):
        b0 = 96 * i - 3
        xv = np.zeros((NT3, 128, WROW), np.float32)
        w2v = np.zeros((NT3, 128, ND, 128), np.float32)
        for tau in range(NT3):
            for rl in range(8):
                rho = b0 + 8 * tau + rl
                if 0 <= rho < HIN:
                    row = x0[:, rho, :]  # [CIN, WIN]
                    xv[tau, rl * 16:rl * 16 + 16, HALO:HALO + WIN] = row
                    xv[tau, rl * 16:rl * 16 + 16, :HALO] = row[:, WIN - HALO:]
                    xv[tau, rl * 16:rl * 16 + 16, HALO + WIN:] = row[:, :HALO]
        h0 = i * HPC2
        h1 = min(HOUT, h0 + HPC2)
        for h in range(h0, h1):
            hl = h - h0
            ms = ((hl + 3) % 8) * 16
            for l in range(NL):
                r = int(lat[h, l]) - b0
                tau, ps_ = r // 8, (r % 8) * 16
                # [ND, C, O] -> [C, ND, O]
                w2v[tau, ps_:ps_ + 16, :, ms:ms + 16] += \
                    W2[h, l].transpose(1, 0, 2)
        xw = np.concatenate(
            [xv, w2v.reshape(NT3, 128, ND * 128)], axis=2)
        in_maps.append({"xw": np.ascontiguousarray(xw).astype(np_dt)})
    return in_maps, use_f32


def _v2_valid(lat):
    """v2 requires each output row's input rows to sit in the 1-2 aligned
    8-row blocks implied by its flush position (true for the structured
    equiangular lat_idx; arbitrary indices fall back to the group scheme)."""
    for i in range(NCORES):
        h0 = i * HPC2
        for h in range(h0, min(HOUT, h0 + HPC2)):
            hl = h - h0
            tf = (hl + 5) // 4
            j = (hl + 5) % 4
            if tf >= NT:
                return False
            gbf = 12 * i - 1 + tf
            blocks = set(int(r) // 8 for r in lat[h])
            allowed = {gbf} if j == 3 else {gbf - 1, gbf}
            if not blocks <= allowed:
                return False
    return True


def _prepare_v2(x, psi, weight, quad_w, lat):
    use_f32 = os.environ.get("KERNEL_DTYPE", "bf16") == "f32"
    np_dt = np.float32 if use_f32 else ml_dtypes.bfloat16

    psi_q = psi.astype(np.float64) * \
        quad_w.astype(np.float64)[lat][None, :, :, None]
    W2 = np.einsum("ock,khld->hldco", weight.astype(np.float64),
                   psi_q).astype(np.float32)

    in_maps = []
    x0 = x[0]
    for i in range(NCORES):
        h0 = i * HPC2
        h1 = min(HOUT, h0 + HPC2)
        xv = np.zeros((NT, 128, WROW), np.float32)
        w2v = np.zeros((NT, 128, ND, 128), np.float32)
        for tau in range(NT):
            gb = 12 * i - 1 + tau
            for rl in range(8):
                rho = 8 * gb + rl
                if 0 <= rho < HIN:
                    row = x0[:, rho, :]  # [CIN, WIN]
                    xv[tau, rl * 16:rl * 16 + 16, HALO:HALO + WIN] = row
                    xv[tau, rl * 16:rl * 16 + 16, :HALO] = row[:, WIN - HALO:]
                    xv[tau, rl * 16:rl * 16 + 16, HALO + WIN:] = row[:, :HALO]
        for h in range(h0, h1):
            hl = h - h0
            ms = ((hl + 1) % 8) * 16
            for l in range(NL):
                rho = int(lat[h, l])
                tau = rho // 8 - (12 * i - 1)
                ps = (rho % 8) * 16
                # [ND, C, O] -> [C, ND, O]
                w2v[tau, ps:ps + 16, :, ms:ms + 16] += \
                    W2[h, l].transpose(1, 0, 2)
        xw = np.concatenate(
            [xv, w2v.reshape(NT, 128, ND * 128)], axis=2)
        in_maps.append({"xw": np.ascontiguousarray(xw).astype(np_dt)})
    return in_maps, use_f32


def _prepare(x, psi, weight, quad_w, lat_idx):
    x = np.asarray(x)
    psi = np.asarray(psi)
    weight = np.asarray(weight)
    quad_w = np.asarray(quad_w)
    lat = np.clip(np.asarray(lat_idx).astype(np.int64), 0, HIN - 1)

    use_f32 = os.environ.get("KERNEL_DTYPE", "bf16") == "f32"
    np_dt = np.float32 if use_f32 else ml_dtypes.bfloat16

    # ---- host fold: W2[h, l, d, c, o]
    psi_q = psi.astype(np.float64) * \
        quad_w.astype(np.float64)[lat][None, :, :, None]
    W2 = np.einsum("ock,khld->hldco", weight.astype(np.float64),
                   psi_q).astype(np.float32)

    # ---- per-core plan (generic in lat_idx; structured input -> RG=21)
    plans = []
    RG = 1
    for i in range(NCORES):
        h0 = i * HPC
        h1 = min(HOUT, h0 + HPC)
        groups = []
        for g in range(NG):
            hs = h0 + g * GRP
            he = min(h1, hs + GRP)
            rows_g = np.unique(lat[hs:he]) if hs < he else np.zeros(
                1, np.int64)
            RG = max(RG, len(rows_g))
            groups.append((hs, he, rows_g))
        plans.append((h0, h1, groups))
    KTOT = RG * CIN
    KT = (KTOT + 127) // 128
    kparts = [min(128, KTOT - kt * 128) for kt in range(KT)]

    # ---- per-core host arrays
    in_maps = []
    x0 = x[0]  # [CIN, HIN, WIN]
    for h0, h1, groups in plans:
        xr = np.zeros((CIN, NG * RG, WIN), np.float32)
        w2h = np.zeros((NG, 128, ND, KT, 128), np.float32)
        for g, (hs, he, rows_g) in enumerate(groups):
            nr = len(rows_g)
            xr[:, g * RG:g * RG + nr, :] = x0[:, rows_g, :]
            for hsub in range(he - hs):
                h = hs + hsub
                js = np.searchsorted(rows_g, lat[h])  # [NL]
                for l in range(NL):
                    j = js[l]
                    q = j * 16
                    # [ND, C, O] -> [C, ND, O]
                    blk = W2[h, l].transpose(1, 0, 2)
                    w2h[g, q % 128:q % 128 + 16, :, q // 128,
                        hsub * 16:hsub * 16 + 16] += blk
        in_maps.append({
            "xr": xr.astype(np_dt),
            "w2": np.ascontiguousarray(
                w2h.reshape(NG, 128, ND * KT * 128)).astype(np_dt),
        })
    return in_maps, plans, RG, KT, kparts, use_f32


def _run(nc, in_maps):
    from concourse.bass_utils import run_bass_kernel_spmd
    trace = os.environ.get("KERNEL_TRACE") == "1"
    try:
        return run_bass_kernel_spmd(nc, in_maps, list(range(NCORES)),
                                    trace=trace)
    except ModuleNotFoundError:
        if not trace:
            raise
        return run_bass_kernel_spmd(nc, in_maps, list(range(NCORES)),
                                    trace=False)


def kernel(x, psi, weight, quad_w, lat_idx):
    global last_result
    import concourse.mybir as mybir
    x = np.asarray(x)
    psi = np.asarray(psi)
    weight = np.asarray(weight)
    quad_w = np.asarray(quad_w)
    lat = np.clip(np.asarray(lat_idx).astype(np.int64), 0, HIN - 1)

    scheme = os.environ.get("KERNEL_SCHEME", "auto")
    use_v3 = scheme == "v3" or (scheme == "auto" and _v3_valid(lat))
    use_v2 = not use_v3 and scheme != "v1" and (
        scheme == "v2" or _v2_valid(lat))

    if use_v3:
        in_maps, use_f32 = _prepare_v3(x, psi, weight, quad_w, lat)
        dt_in = mybir.dt.float32 if use_f32 else mybir.dt.bfloat16
        warmup = os.environ.get("KERNEL_WARMUP", "1") == "1"
        key = ("v3", str(dt_in), warmup)
        if key not in _cache:
            _cache[key] = _build_nc_v3(dt_in, warmup)
        res = _run(_cache[key], in_maps)
        last_result = res
        out = np.empty((B, COUT, HOUT, WOUT), np.float32)
        for i in range(NCORES):
            h0 = i * HPC2
            h1 = min(HOUT, h0 + HPC2)
            out[0, :, h0:h1, :] = res.results[i]["y"][3:3 + h1 - h0] \
                .astype(np.float32).transpose(1, 0, 2)
        return out.astype(x.dtype)

    if use_v2:
        in_maps, use_f32 = _prepare_v2(x, psi, weight, quad_w, lat)
        dt_in = mybir.dt.float32 if use_f32 else mybir.dt.bfloat16
        key = ("v2", str(dt_in))
        if key not in _cache:
            _cache[key] = _build_nc_v2(dt_in)
        res = _run(_cache[key], in_maps)
        last_result = res
        out = np.empty((B, COUT, HOUT, WOUT), np.float32)
        for i in range(NCORES):
            h0 = i * HPC2
            h1 = min(HOUT, h0 + HPC2)
            out[0, :, h0:h1, :] = \
                res.results[i]["y"][5:5 + h1 - h0].transpose(1, 0, 2)
        return out.astype(x.dtype)

    in_maps, plans, RG, KT, kparts, use_f32 = _prepare(
        x, psi, weight, quad_w, lat_idx)
    dt_in = mybir.dt.float32 if use_f32 else mybir.dt.bfloat16
    key = (RG, KT, tuple(kparts), str(dt_in))
    if key not in _cache:
        _cache[key] = _build_nc(RG, KT, kparts, dt_in)
    res = _run(_cache[key], in_maps)
    last_result = res

    out = np.empty((B, COUT, HOUT, WOUT), np.float32)
    for i, (h0, h1, _) in enumerate(plans):
        out[0, :, h0:h1, :] = res.results[i]["y"][:, :h1 - h0, :]
    return out.astype(x.dtype)



# revision 20
# speedup vs baseline: 1.0265x; 1.0265x over previous
"""DISCO (discrete-continuous) spherical conv encoder on 8 Trainium2 cores.

Strategy: output-latitude sharding (361 rows -> ~46/core), no collectives.
Host folds weight[o,c,k] x psi[k,h,l,d] x quad_w[lat_idx[h,l]] into per-h
matmul coefficients; device does per-latitude-group matmuls with PSUM
accumulation over the 9 longitude shifts (stride-2 rhs APs give the
PSCALE=2 decimation for free; a 4-col halo handles the longitude wrap).
"""
import os
import numpy as np
import ml_dtypes

B, CIN, COUT = 1, 16, 16
HIN, WIN = 721, 1440
HOUT, WOUT = 361, 720
KBAS, NL, ND = 9, 7, 9
NCORES = 8
HPC = 46          # valid output rows per core (last core: 39)
GRP = 8           # output rows per group
NG = 6            # groups per core (8*6=48 slots >= 46)
HBLK = NG * GRP   # 48
HALO = ND // 2    # 4
WROW = WIN + 2 * HALO  # 1448
NCHUNKS = ((0, 512), (512, WOUT - 512))  # psum-bank-aligned N split
# v3 order: short chunk first so the pulled-ahead LDWEIGHTS of the next
# (tau, d) group (~99ns) hides fully under the long 512-col matmul
NCHUNKS3 = ((512, WOUT - 512), (0, 512))

# ---- v2 sliding-window scheme constants
HPC2 = 48         # output rows per core (48*8=384 >= 361); 48%8==0 keeps the
                  # block->slot mapping identical on every core (SPMD)
NT = 14           # aligned 8-input-row blocks per core (incl leading dummy)
NPS = 4           # rotating PSUM accumulators (4 x 2 banks = all 8 banks)

# ---- v3: like v2 but blocks offset so no dummy block is needed
NT3 = 13          # 8-input-row blocks per core, base input row 96*i - 3

_cache = {}
last_result = None


def _build_nc(RG, KT, kparts, dt_in):
    import concourse.bass as bass
    import concourse.bacc as bacc
    import concourse.mybir as mybir
    from concourse import tile

    nc = bacc.Bacc("TRN2", target_bir_lowering=False, debug=False,
                   num_devices=NCORES)
    xr = nc.declare_dram_parameter("xr", [CIN, NG * RG, WIN], dt_in,
                                   isOutput=False)
    w2 = nc.declare_dram_parameter("w2", [NG, 128, ND * KT * 128], dt_in,
                                   isOutput=False)
    y = nc.declare_dram_parameter("y", [COUT, HBLK, WOUT], mybir.dt.float32,
                                  isOutput=True)
    xr_t = xr.ap().transpose([1, 0, 2])  # [row, c, w]
    y_t = y.ap().transpose([1, 0, 2])    # [h, o, w]

    with tile.TileContext(nc) as tc:
        with (
            tc.tile_pool(name="rbp", bufs=2) as rbp,
            tc.tile_pool(name="w2p", bufs=2) as w2p,
            tc.tile_pool(name="psp", bufs=2, space="PSUM") as psp,
            tc.tile_pool(name="outp", bufs=2) as outp,
        ):
            for g in range(NG):
                w2t = w2p.tile([128, ND * KT * 128], dt_in, tag="w2")
                nc.sync.dma_start(out=w2t[:, :], in_=w2.ap()[g])
                rbs = []
                for kt in range(KT):
                    nrows = (kparts[kt] + 15) // 16  # rows in this k-tile
                    rb = rbp.tile([128, WROW], dt_in, tag=f"rb{kt}")
                    np_ = nrows * CIN
                    r0 = g * RG + 8 * kt  # first xr row of this tile
                    # body + wrap halos; partitions = (row, c), src 3D
                    for dst_c0, src_c0, ncol in (
                        (HALO, 0, WIN),
                        (0, WIN - HALO, HALO),
                        (HALO + WIN, 0, HALO),
                    ):
                        nc.sync.dma_start(
                            out=rb[0:np_, dst_c0:dst_c0 + ncol],
                            in_=xr_t[r0:r0 + nrows, :, src_c0:src_c0 + ncol])
                    rbs.append(rb)
                pss = [psp.tile([128, nw], mybir.dt.float32, tag=f"ps{ci}",
                                name=f"ps{ci}_{g}")
                       for ci, (w0, nw) in enumerate(NCHUNKS)]
                for d in range(ND):
                    for kt in range(KT):
                        kp = kparts[kt]
                        lhsT = w2t[0:kp, (d * KT + kt) * 128:
                                   (d * KT + kt) * 128 + 128]
                        first = d == 0 and kt == 0
                        last = d == ND - 1 and kt == KT - 1
                        for ci, (w0, nw) in enumerate(NCHUNKS):
                            c0 = d + 2 * w0
                            nc.tensor.matmul(
                                pss[ci][:, :], lhsT,
                                rbs[kt][0:kp, c0:c0 + 2 * nw:2],
                                start=first, stop=last)
                stage = outp.tile([128, WOUT], mybir.dt.float32, tag="stage",
                                  name=f"stage_{g}")
                for ci, (w0, nw) in enumerate(NCHUNKS):
                    nc.vector.tensor_copy(out=stage[:, w0:w0 + nw],
                                          in_=pss[ci][:, :])
                nc.sync.dma_start(
                    out=y_t[g * GRP:(g + 1) * GRP, :, :], in_=stage[:, :])
    nc.compile()
    return nc


def _build_nc_v2(dt_in):
    """Sliding-window scheme: one K=128 matmul tile per aligned 8-input-row
    block x 9 lon shifts, accumulating into a rotating set of 4 PSUM tiles
    with output slot = (local output row) mod 8.  After block tau, output
    rows 4*tau-5 .. 4*tau-2 (local) are complete: rows spanning two blocks
    are summed from two PSUM tiles on the Vector engine, single-block rows
    are copied, and the result is DMA'd to y rows [4*tau, 4*tau+4)."""
    import concourse.bacc as bacc
    import concourse.mybir as mybir
    from concourse import tile

    nc = bacc.Bacc("TRN2", target_bir_lowering=False, debug=False,
                   num_devices=NCORES)
    xw = nc.declare_dram_parameter("xw", [NT, 128, WROW + ND * 128],
                                   dt_in, isOutput=False)
    y = nc.declare_dram_parameter("y", [4 * NT, COUT, WOUT],
                                  mybir.dt.float32, isOutput=True)

    with tile.TileContext(nc) as tc:
        with (
            tc.tile_pool(name="xbp", bufs=3) as xbp,
            tc.tile_pool(name="psp", bufs=1, space="PSUM") as psp,
            tc.tile_pool(name="stp", bufs=4) as stp,
        ):
            P = [psp.tile([128, WOUT], mybir.dt.float32, tag=f"P{j}",
                          name=f"P{j}") for j in range(NPS)]
            for tau in range(NT):
                xb = xbp.tile([128, WROW + ND * 128], dt_in, tag="xb",
                              name=f"xb{tau}")
                # two issues -> two HW queues -> ~half the load latency
                nc.sync.dma_start(out=xb[0:64, :], in_=xw.ap()[tau][0:64])
                nc.sync.dma_start(out=xb[64:128, :], in_=xw.ap()[tau][64:128])
                ps = P[tau % NPS]
                for d in range(ND):
                    lhsT = xb[:, WROW + d * 128:WROW + (d + 1) * 128]
                    for w0, nw in NCHUNKS:
                        c0 = d + 2 * w0
                        nc.tensor.matmul(ps[:, w0:w0 + nw], lhsT,
                                         xb[:, c0:c0 + 2 * nw:2],
                                         start=(d == 0), stop=(d == ND - 1))
                # flush the 4 output rows finished by this block; with
                # slot = (h_local+1)%8 the flush set is one 64-partition
                # range at base 64 (even tau) / 0 (odd tau).  The j==3
                # (single-block) row's prev-tile contribution is exactly 0
                # (its weights there are zero), so one add covers all 4.
                st = stp.tile([128, WOUT], mybir.dt.float32, tag="st",
                              name=f"st{tau}")
                prev = P[(tau - 1) % NPS]
                p0 = 64 if tau % 2 == 0 else 0
                if tau == 0:
                    # dummy flush: no valid rows; keep the program uniform
                    nc.vector.tensor_copy(out=st[p0:p0 + 64, :],
                                          in_=ps[p0:p0 + 64, :])
                else:
                    # DVE reads at most one PSUM operand: stage prev via the
                    # (otherwise idle) Scalar engine.  The copy only depends
                    # on tile tau-1, so it overlaps this tile's matmuls.
                    cp = stp.tile([128, WOUT], mybir.dt.float32, tag="cp",
                                  name=f"cp{tau}")
                    nc.scalar.copy(out=cp[p0:p0 + 64, :],
                                   in_=prev[p0:p0 + 64, :])
                    nc.vector.tensor_add(out=st[p0:p0 + 64, :],
                                         in0=cp[p0:p0 + 64, :],
                                         in1=ps[p0:p0 + 64, :])
                nc.gpsimd.dma_start(out=y.ap()[4 * tau:4 * tau + 4],
                                    in_=st[p0:p0 + 64, :])
    # The tile exit splits each matmul into InstLdweights + InstMatmult.
    # The two N-chunk matmuls of each (tau, d) share a stationary operand:
    # drop the second (redundant) load when it carries no semaphore ops.
    nskip = 0
    for bb in nc.m.functions[0].blocks:
        prev_w = None
        dels = []
        for inst in bb.instructions:
            if type(inst).__name__ != "InstLdweights":
                continue
            w = str(inst.ins[0])
            if w == prev_w and not inst.has_wait() and not inst.has_update():
                dels.append(inst)
            prev_w = w
        for inst in dels:
            bb.instructions.remove(inst)
        nskip += len(dels)
    assert nskip >= NT * ND - 8, \
        f"expected ~{NT * ND} redundant LDWEIGHTS, got {nskip}"
    nc.compile()
    return nc


def _build_nc_v3(dt_in, warmup=True):
    """v3 sliding-window scheme: per-core input rows start at 96*i-3 so
    block tau's 8 rows serve exactly output rows hl in [4t-3, 4t+3]; the
    flush after tau covers hl in [4t-3, 4t] at slot (hl+3)%8 (one
    64-partition half, base 0 for even tau / 64 for odd).  Row 4t lies
    entirely in block tau, rows 4t-3..4t-1 straddle tau-1/tau and are
    summed from two PSUM tiles.  tau=0's flush has no prev tile and row 0
    (slot 3) is single-block, so it is a plain copy."""
    import concourse.bacc as bacc
    import concourse.mybir as mybir
    from concourse import tile

    nc = bacc.Bacc("TRN2", target_bir_lowering=False, debug=False,
                   num_devices=NCORES)
    xw = nc.declare_dram_parameter("xw", [NT3, 128, WROW + ND * 128],
                                   dt_in, isOutput=False)
    y = nc.declare_dram_parameter("y", [4 * NT3, COUT, WOUT],
                                  mybir.dt.bfloat16, isOutput=True)

    with tile.TileContext(nc) as tc:
        with (
            tc.tile_pool(name="xbp", bufs=NT3) as xbp,
            tc.tile_pool(name="psp", bufs=1, space="PSUM") as psp,
            # separate pools so st/cp each rotate over their own buffers
            # (shared-pool slots would halve the WAR reuse distance)
            tc.tile_pool(name="stp", bufs=5) as stp,
            tc.tile_pool(name="cpp", bufs=5) as cpp,
            tc.tile_pool(name="wup", bufs=1) as wup,
        ):
            P = [psp.tile([128, WOUT], mybir.dt.float32, tag=f"P{j}",
                          name=f"P{j}") for j in range(NPS)]
            if warmup:
                # HAM warmup: PE activity from program start until the
                # first x block lands (~11.4us) so the clock gate opens at
                # start+3.4us and never resets (an idle gap restarts the
                # busy window and the first real blocks run at 1.2GHz).
                wz = wup.tile([128, 640], dt_in, tag="wz")
                nc.gpsimd.memset(wz[:, :], 0.0)
                for i in range(12):
                    nc.tensor.matmul(P[NPS - 1][:, 0:512], wz[:, 0:128],
                                     wz[:, 128:640], start=True, stop=True)
            # Hoist all x-loads: Tile round-robins 8 DMA-completion sem
            # lanes over every DMA in program order; interleaving loads
            # with y-stores makes load dispatches wait on store completions
            # (which depend on the flush chain).  Issued back-to-back, the
            # loads only lane-wait on each other.
            xbs = []
            for tau in range(NT3):
                xb = xbp.tile([128, WROW + ND * 128], dt_in, tag="xb",
                              name=f"xb{tau}")
                if tau == 0:
                    # split so the d=0/d=1 groups start ~0.5us earlier
                    c1 = WROW + 2 * 128
                    nc.sync.dma_start(out=xb[:, 0:c1],
                                      in_=xw.ap()[tau][:, 0:c1])
                    nc.sync.dma_start(out=xb[:, c1:],
                                      in_=xw.ap()[tau][:, c1:])
                else:
                    nc.sync.dma_start(out=xb[:, :], in_=xw.ap()[tau])
                xbs.append(xb)
            for tau in range(NT3):
                xb = xbs[tau]
                ps = P[tau % NPS]
                for d in range(ND):
                    lhsT = xb[:, WROW + d * 128:WROW + (d + 1) * 128]
                    # zigzag: consecutive matmuls hit the same PSUM bank
                    # across group boundaries (bank switches cause PE
                    # micro-idles), and even-d order ends on the 512 chunk
                    # so the pulled-ahead LDWEIGHTS hides under it
                    chunks = NCHUNKS3 if d % 2 == 0 else NCHUNKS3[::-1]
                    for w0, nw in chunks:
                        c0 = d + 2 * w0
                        nc.tensor.matmul(ps[:, w0:w0 + nw], lhsT,
                                         xb[:, c0:c0 + 2 * nw:2],
                                         start=(d == 0), stop=(d == ND - 1))
                st = stp.tile([128, WOUT], mybir.dt.bfloat16, tag="st",
                              name=f"st{tau}")
                p0 = 0 if tau % 2 == 0 else 64
                nrow = 3 if tau == NT3 - 1 else 4  # hl=4*12 is never valid
                npart = 16 * nrow
                if tau == 0:
                    nc.scalar.copy(out=st[p0:p0 + npart, :],
                                   in_=ps[p0:p0 + npart, :])
                elif tau < NT3 - 1:
                    # ACT stages the CURRENT block's psum right after its
                    # matmuls; DVE adds the previous block's psum.  This
                    # frees P[tau%4] for block tau+4 via the immediate
                    # copy - the late reader (the add) touches P[(tau-1)%4]
                    # whose next writer is block tau+3, keeping the PSUM
                    # WAR release ~3 blocks ahead of the PE.
                    cp = cpp.tile([128, WOUT], mybir.dt.float32, tag="cp",
                                  name=f"cp{tau}")
                    prev = P[(tau - 1) % NPS]
                    nc.scalar.copy(out=cp[p0:p0 + npart, :],
                                   in_=ps[p0:p0 + npart, :])
                    nc.vector.tensor_add(out=st[p0:p0 + npart, :],
                                         in0=cp[p0:p0 + npart, :],
                                         in1=prev[p0:p0 + npart, :])
                else:
                    # last flush: stage the prev tile early (during this
                    # block's matmuls), and do the [512:720] bank first -
                    # its psum chunk is complete before the final 512-col
                    # matmul, so that add+store overlap the last matmul
                    cp = cpp.tile([128, WOUT], mybir.dt.float32, tag="cp",
                                  name=f"cp{tau}")
                    prev = P[(tau - 1) % NPS]
                    nc.scalar.copy(out=cp[p0:p0 + npart, :],
                                   in_=prev[p0:p0 + npart, :])
                    for w0, nw in ((512, WOUT - 512), (0, 512)):
                        nc.vector.tensor_add(
                            out=st[p0:p0 + npart, w0:w0 + nw],
                            in0=cp[p0:p0 + npart, w0:w0 + nw],
                            in1=ps[p0:p0 + npart, w0:w0 + nw])
                        nc.scalar.dma_start(
                            out=y.ap()[4 * tau:4 * tau + nrow]
                            [:, :, w0:w0 + nw],
                            in_=st[p0:p0 + npart, w0:w0 + nw])
                if tau < NT3 - 1:
                    nc.scalar.dma_start(out=y.ap()[4 * tau:4 * tau + nrow],
                                        in_=st[p0:p0 + npart, :])
    _dedup_ldweights(nc, NT3 * ND - 8)
    if os.environ.get("KERNEL_THIN", "1") == "1":
        _thin_pe_progress_sem(nc)
    if warmup and os.environ.get("KERNEL_HOIST", "1") == "1":
        _hoist_startup(nc)
    nc.compile()
    return nc


def _hoist_startup(nc, n_dma=3):
    """Move the first n_dma x-load dispatches and the warmup matmuls from
    the main block into the preamble block.  The Sync engine otherwise
    idles ~7us in preamble barriers before dispatching any DMA, and the
    PE exits the preamble cold.  Hoisted, the first block's data is in
    SBUF before the preamble barrier completes and the PE is HAM-warm.
    The warmup reads garbage SBUF (its output PSUM tile is overwritten
    with start=True later), so its memset dependency is stripped.  Only
    n_dma dispatches move: each costs ~600ns on the Sync queue and more
    would delay Sync's arrival at the preamble barrier."""
    import bass_rust

    f = nc.m.functions[0]
    b0, b1 = f.blocks[0], f.blocks[1]
    moved = []
    ndma = 0
    for inst in list(b1.instructions):
        tn = type(inst).__name__
        if tn == "InstDMACopy" and "SP" in str(inst.engine):
            si = inst.sync_info
            assert not (si and si.on_wait), "first loads must be wait-free"
            b1.instructions.remove(inst)
            moved.append(inst)
            ndma += 1
            if ndma >= n_dma:
                break
    pe_moved = []
    nmm = 0
    for inst in list(b1.instructions):
        tn = type(inst).__name__
        if tn not in ("InstLdweights", "InstMatmult"):
            continue
        si = inst.sync_info
        if si is not None and (si.on_wait or si.on_update):
            assert not si.on_update, "warmup matmuls should carry no incs"
            inst.sync_info = bass_rust.SyncInfo(
                on_wait=[], on_update=list(si.on_update))
        b1.instructions.remove(inst)
        pe_moved.append(inst)
        if tn == "InstMatmult":
            nmm += 1
            if nmm >= 12:
                break  # the 12 warmup matmuls (+ their LDW) only
    assert ndma == n_dma and nmm == 12, (ndma, nmm)
    for inst in reversed(moved + pe_moved):
        b0.instructions.insert(0, inst)


def _thin_pe_progress_sem(nc):
    """Every matmul carries a +1 on the Tile PE-progress semaphore (~26ns
    of serialized EVT-register writes each).  Consumers only wait at a few
    thresholds, and a FIFO engine completes in order, so "first n done" ==
    "n-th done": keep the inc only on matmuls whose position is a waited
    threshold and renumber the waits to the threshold's rank."""
    import bass_rust

    f = nc.m.functions[0]
    from collections import Counter

    upd = Counter()
    for bb in f.blocks:
        for inst in bb.instructions:
            if type(inst).__name__ != "InstMatmult":
                continue
            si = inst.sync_info
            if si is None:
                continue
            for u in si.on_update:
                if u.update_mode == "sem-inc" and u.update_value == 1:
                    upd[u.id] += 1
    if not upd:
        return
    sem_id, n_mm = upd.most_common(1)[0]
    thresholds = set()
    ok = True
    for bb in f.blocks:
        for inst in bb.instructions:
            si = inst.sync_info
            if si is None:
                continue
            for w in si.on_wait:
                if w.id == sem_id:
                    if w.wait_mode != "sem-ge-imm" or w.wait_reg is not None:
                        ok = False
                    thresholds.add(w.wait_value)
    if not ok or not thresholds or max(thresholds) > n_mm:
        return
    T = sorted(thresholds)
    rank = {t: j + 1 for j, t in enumerate(T)}
    n = 0
    for bb in f.blocks:
        for inst in bb.instructions:
            si = inst.sync_info
            if si is None:
                continue
            touched = False
            new_upd = []
            for u in si.on_update:
                if (type(inst).__name__ == "InstMatmult"
                        and u.id == sem_id and u.update_mode == "sem-inc"):
                    n += 1
                    if n not in thresholds:
                        touched = True
                        continue  # drop the inc
                new_upd.append(u)
            new_wait = []
            for w in si.on_wait:
                if w.id == sem_id:
                    touched = True
                    w = bass_rust.SyncWait(
                        sync_type=w.sync_type, id=w.id, ant_name=w.ant_name,
                        wait_mode=w.wait_mode, wait_value=rank[w.wait_value])
                new_wait.append(w)
            if touched:
                inst.sync_info = bass_rust.SyncInfo(
                    on_wait=new_wait, on_update=new_upd)
    assert n == n_mm, (n, n_mm)


def _dedup_ldweights(nc, min_expected):
    """Drop back-to-back InstLdweights with an identical stationary
    operand (the Tile exit emits one per matmul; N-chunk pairs share)."""
    nskip = 0
    for bb in nc.m.functions[0].blocks:
        prev_w = None
        dels = []
        for inst in bb.instructions:
            if type(inst).__name__ != "InstLdweights":
                continue
            w = str(inst.ins[0])
            if w == prev_w and not inst.has_wait() and not inst.has_update():
                dels.append(inst)
            prev_w = w
        for inst in dels:
            bb.instructions.remove(inst)
        nskip += len(dels)
    assert nskip >= min_expected, \
        f"expected >= {min_expected} redundant LDWEIGHTS, got {nskip}"


def _v3_valid(lat):
    """v3 requires each output row's input rows to lie in the 1-2 blocks
    implied by its flush position (true for the structured equiangular
    lat_idx; arbitrary indices fall back to the generic group scheme)."""
    for i in range(NCORES):
        h0 = i * HPC2
        b0 = 96 * i - 3
        for h in range(h0, min(HOUT, h0 + HPC2)):
            hl = h - h0
            tf = (hl + 3) // 4
            rs = lat[h].astype(np.int64) - b0
            if rs.min() < 0 or rs.max() >= 8 * NT3:
                return False
            blocks = set(int(r) // 8 for r in rs)
            allowed = {tf} if tf == 0 else {tf - 1, tf}
            if not blocks <= allowed:
                return False
    return True


def _prepare_v3(x, psi, weight, quad_w, lat):
    use_f32 = os.environ.get("KERNEL_DTYPE", "bf16") == "f32"
    np_dt = np.float32 if use_f32 else ml_dtypes.bfloat16

    psi_q = psi.astype(np.float64) * \
        quad_w.astype(np.float64)[lat][None, :, :, None]
    W2 = np.einsum("ock,khld->hldco", weight.astype(np.float64),
                   psi_q).astype(np.float32)

    in_maps = []
    x0 = x[0]
    for i in range(NCORES):
        b0 = 96 * i - 3
        xv = np.zeros((NT3, 128, WROW), np.float32)
        w2v = np.zeros((NT3, 128, ND, 128), np.float32)
        for tau in range(NT3):
            for rl in range(8):
                rho = b0 + 8 * tau + rl
                if 0 <= rho < HIN:
                    row = x0[:, rho, :]  # [CIN, WIN]
                    xv[tau, rl * 16:rl * 16 + 16, HALO:HALO + WIN] = row
                    xv[tau, rl * 16:rl * 16 + 16, :HALO] = row[:, WIN - HALO:]
                    xv[tau, rl * 16:rl * 16 + 16, HALO + WIN:] = row[:, :HALO]
        h0 = i * HPC2
        h1 = min(HOUT, h0 + HPC2)
        for h in range(h0, h1):
            hl = h - h0
            ms = ((hl + 3) % 8) * 16
            for l in range(NL):
                r = int(lat[h, l]) - b0
                tau, ps_ = r // 8, (r % 8) * 16
                # [ND, C, O] -> [C, ND, O]
                w2v[tau, ps_:ps_ + 16, :, ms:ms + 16] += \
                    W2[h, l].transpose(1, 0, 2)
        xw = np.concatenate(
            [xv, w2v.reshape(NT3, 128, ND * 128)], axis=2)
        in_maps.append({"xw": np.ascontiguousarray(xw).astype(np_dt)})
    return in_maps, use_f32


def _v2_valid(lat):
    """v2 requires each output row's input rows to sit in the 1-2 aligned
    8-row blocks implied by its flush position (true for the structured
    equiangular lat_idx; arbitrary indices fall back to the group scheme)."""
    for i in range(NCORES):
        h0 = i * HPC2
        for h in range(h0, min(HOUT, h0 + HPC2)):
            hl = h - h0
            tf = (hl + 5) // 4
            j = (hl + 5) % 4
            if tf >= NT:
                return False
            gbf = 12 * i - 1 + tf
            blocks = set(int(r) // 8 for r in lat[h])
            allowed = {gbf} if j == 3 else {gbf - 1, gbf}
            if not blocks <= allowed:
                return False
    return True


def _prepare_v2(x, psi, weight, quad_w, lat):
    use_f32 = os.environ.get("KERNEL_DTYPE", "bf16") == "f32"
    np_dt = np.float32 if use_f32 else ml_dtypes.bfloat16

    psi_q = psi.astype(np.float64) * \
        quad_w.astype(np.float64)[lat][None, :, :, None]
    W2 = np.einsum("ock,khld->hldco", weight.astype(np.float64),
                   psi_q).astype(np.float32)

    in_maps = []
    x0 = x[0]
    for i in range(NCORES):
        h0 = i * HPC2
        h1 = min(HOUT, h0 + HPC2)
        xv = np.zeros((NT, 128, WROW), np.float32)
        w2v = np.zeros((NT, 128, ND, 128), np.float32)
        for tau in range(NT):
            gb = 12 * i - 1 + tau
            for rl in range(8):
                rho = 8 * gb + rl
                if 0 <= rho < HIN:
                    row = x0[:, rho, :]  # [CIN, WIN]
                    xv[tau, rl * 16:rl * 16 + 16, HALO:HALO + WIN] = row
                    xv[tau, rl * 16:rl * 16 + 16, :HALO] = row[:, WIN - HALO:]
                    xv[tau, rl * 16:rl * 16 + 16, HALO + WIN:] = row[:, :HALO]
        for h in range(h0, h1):
            hl = h - h0
            ms = ((hl + 1) % 8) * 16
            for l in range(NL):
                rho = int(lat[h, l])
                tau = rho // 8 - (12 * i - 1)
                ps = (rho % 8) * 16
                # [ND, C, O] -> [C, ND, O]
                w2v[tau, ps:ps + 16, :, ms:ms + 16] += \
                    W2[h, l].transpose(1, 0, 2)
        xw = np.concatenate(
            [xv, w2v.reshape(NT, 128, ND * 128)], axis=2)
        in_maps.append({"xw": np.ascontiguousarray(xw).astype(np_dt)})
    return in_maps, use_f32


def _prepare(x, psi, weight, quad_w, lat_idx):
    x = np.asarray(x)
    psi = np.asarray(psi)
    weight = np.asarray(weight)
    quad_w = np.asarray(quad_w)
    lat = np.clip(np.asarray(lat_idx).astype(np.int64), 0, HIN - 1)

    use_f32 = os.environ.get("KERNEL_DTYPE", "bf16") == "f32"
    np_dt = np.float32 if use_f32 else ml_dtypes.bfloat16

    # ---- host fold: W2[h, l, d, c, o]
    psi_q = psi.astype(np.float64) * \
        quad_w.astype(np.float64)[lat][None, :, :, None]
    W2 = np.einsum("ock,khld->hldco", weight.astype(np.float64),
                   psi_q).astype(np.float32)

    # ---- per-core plan (generic in lat_idx; structured input -> RG=21)
    plans = []
    RG = 1
    for i in range(NCORES):
        h0 = i * HPC
        h1 = min(HOUT, h0 + HPC)
        groups = []
        for g in range(NG):
            hs = h0 + g * GRP
            he = min(h1, hs + GRP)
            rows_g = np.unique(lat[hs:he]) if hs < he else np.zeros(
                1, np.int64)
            RG = max(RG, len(rows_g))
            groups.append((hs, he, rows_g))
        plans.append((h0, h1, groups))
    KTOT = RG * CIN
    KT = (KTOT + 127) // 128
    kparts = [min(128, KTOT - kt * 128) for kt in range(KT)]

    # ---- per-core host arrays
    in_maps = []
    x0 = x[0]  # [CIN, HIN, WIN]
    for h0, h1, groups in plans:
        xr = np.zeros((CIN, NG * RG, WIN), np.float32)
        w2h = np.zeros((NG, 128, ND, KT, 128), np.float32)
        for g, (hs, he, rows_g) in enumerate(groups):
            nr = len(rows_g)
            xr[:, g * RG:g * RG + nr, :] = x0[:, rows_g, :]
            for hsub in range(he - hs):
                h = hs + hsub
                js = np.searchsorted(rows_g, lat[h])  # [NL]
                for l in range(NL):
                    j = js[l]
                    q = j * 16
                    # [ND, C, O] -> [C, ND, O]
                    blk = W2[h, l].transpose(1, 0, 2)
                    w2h[g, q % 128:q % 128 + 16, :, q // 128,
                        hsub * 16:hsub * 16 + 16] += blk
        in_maps.append({
            "xr": xr.astype(np_dt),
            "w2": np.ascontiguousarray(
                w2h.reshape(NG, 128, ND * KT * 128)).astype(np_dt),
        })
    return in_maps, plans, RG, KT, kparts, use_f32


def _run(nc, in_maps):
    from concourse.bass_utils import run_bass_kernel_spmd
    trace = os.environ.get("KERNEL_TRACE") == "1"
    try:
        return run_bass_kernel_spmd(nc, in_maps, list(range(NCORES)),
                                    trace=trace)
    except ModuleNotFoundError:
        if not trace:
            raise
        return run_bass_kernel_spmd(nc, in_maps, list(range(NCORES)),
                                    trace=False)


def kernel(x, psi, weight, quad_w, lat_idx):
    global last_result
    import concourse.mybir as mybir
    x = np.asarray(x)
    psi = np.asarray(psi)
    weight = np.asarray(weight)
    quad_w = np.asarray(quad_w)
    lat = np.clip(np.asarray(lat_idx).astype(np.int64), 0, HIN - 1)

    scheme = os.environ.get("KERNEL_SCHEME", "auto")
    use_v3 = scheme == "v3" or (scheme == "auto" and _v3_valid(lat))
    use_v2 = not use_v3 and scheme != "v1" and (
        scheme == "v2" or _v2_valid(lat))

    if use_v3:
        in_maps, use_f32 = _prepare_v3(x, psi, weight, quad_w, lat)
        dt_in = mybir.dt.float32 if use_f32 else mybir.dt.bfloat16
        warmup = os.environ.get("KERNEL_WARMUP", "1") == "1"
        key = ("v3", str(dt_in), warmup)
        if key not in _cache:
            _cache[key] = _build_nc_v3(dt_in, warmup)
        res = _run(_cache[key], in_maps)
        last_result = res
        out = np.empty((B, COUT, HOUT, WOUT), np.float32)
        for i in range(NCORES):
            h0 = i * HPC2
            h1 = min(HOUT, h0 + HPC2)
            out[0, :, h0:h1, :] = res.results[i]["y"][3:3 + h1 - h0] \
                .astype(np.float32).transpose(1, 0, 2)
        return out.astype(x.dtype)

    if use_v2:
        in_maps, use_f32 = _prepare_v2(x, psi, weight, quad_w, lat)
        dt_in = mybir.dt.float32 if use_f32 else mybir.dt.bfloat16
        key = ("v2", str(dt_in))
        if key not in _cache:
            _cache[key] = _build_nc_v2(dt_in)
        res = _run(_cache[key], in_maps)
        last_result = res
        out = np.empty((B, COUT, HOUT, WOUT), np.float32)
        for i in range(NCORES):
            h0 = i * HPC2
            h1 = min(HOUT, h0 + HPC2)
            out[0, :, h0:h1, :] = \
                res.results[i]["y"][5:5 + h1 - h0].transpose(1, 0, 2)
        return out.astype(x.dtype)

    in_maps, plans, RG, KT, kparts, use_f32 = _prepare(
        x, psi, weight, quad_w, lat_idx)
    dt_in = mybir.dt.float32 if use_f32 else mybir.dt.bfloat16
    key = (RG, KT, tuple(kparts), str(dt_in))
    if key not in _cache:
        _cache[key] = _build_nc(RG, KT, kparts, dt_in)
    res = _run(_cache[key], in_maps)
    last_result = res

    out = np.empty((B, COUT, HOUT, WOUT), np.float32)
    for i, (h0, h1, _) in enumerate(plans):
        out[0, :, h0:h1, :] = res.results[i]["y"][:, :h1 - h0, :]
    return out.astype(x.dtype)

